# revision 43
# baseline (speedup 1.0000x reference)
"""MoE soft-routing MLP kernel for 8 Trainium2 NeuronCores.

Reference computation (per layer l, weights a_l: [E, out, in], bias b_l: [E, out]):
    y_e = H @ a_e^T + b_e          # per-expert GEMM      [B, out]
    H'  = sum_e wb[e, :, None] * y_e                      [B, out]
    H'  = elu(H') for layers 0, 1

Distribution: data-parallel over batch B=4096 across 8 cores (B_loc=512).
Expert weights are replicated to every core; x and weight_blend are sharded
along batch.

Per-core algorithm (all activations kept TRANSPOSED on chip: [feature, batch]):
    out[o, b] = sum_e sum_i aT_e[i, o] * (wb[e, b] * Ht[i, b])  + bias term
  - each expert's contribution accumulates into the same PSUM bank:
    lhsT = aT_e[i-tile, o-chunk] (128x128 stationary),
    rhs  = Zt_e[i-tile] = Ht[i-tile] * bcast(wb[e, :]) (128x512 moving),
  - blend weights arrive pre-broadcast from the host ([E, 128, B_LOC] fp16).
  - ELU+1 is evicted as relu(x) + min(exp(x), 1)  (= elu(x) + 1; valid since
    the preactivations here are far below exp-overflow), and the -1 folds
    into the next layer's blend: zt = (h1 - 1) * wbb_e, one DVE op.
    (ht must stay fp32: elu values are ~1e-2 around the +1 offset, so fp16
    storage of elu+1 would cost ~3% relative error after the -1.)

Matmul operands are fp16 with fp32 PSUM accumulation (1 PE cycle/row and
half the weight-DMA bytes of fp32; weights are pre-scaled by 2^8 and blend
weights by 2^6 on the host so fp16 products stay clear of the subnormal
range; the 2^-14 descale folds into the PSUM-eviction activations).
PE floor: 1024 matmuls x ~216ns = ~221us; DMA total ~20MB/core.

Scheduling design (from trace analysis; measured ~239.6us/core vs a
~221.4us pure-PE floor, with ZERO PE gaps in steady state):
  - Weights live in DRAM pre-swizzled to SBUF layout [E, 128, ni*dout]
    (host does the permutation), so ONE dma_start per expert per layer
    moves the whole expert contiguously (8-16KB/partition descriptors) at
    near-HBM rate. ~47 dma_starts total: DMA *issue* time (~0.6us each on
    a sequencer) stops gating the startup.
  - Everything rides the two HWDGE rings (sync / scalar) — the SWDGE
    (gpsimd) path adds ~1us of Q7 descriptor emission, which put the
    blend-weight broadcast on the v2 critical path. Startup-critical order:
    sync: wbb[e0], w[e0] j0 split 2:6, j1..j3, then (wbb[e], w[e]) pairs;
    scalar: the four xt j-tiles, later the output stores.
  - x and the blend broadcast are fp16 (x is N(0,1): 5e-4 quantization,
    same as the matmul operands already pay). Output is stored fp16 scaled
    by 2^12 (values ~0.27) and un-scaled on the host: halves the tail DMA.
  - 9 junk matmuls bridge the ~4.5us from kernel start to the first
    operand landing (HWDGE issue + HBM read + write-receipt ~4us for the
    first tiles) and warm the PE HAM clock gate (~3.4us sustained activity
    -> 2.4GHz) so the real MMs run warm from the first one.
  - The last-expert pass of each layer runs c-outer so PSUM bank closures
    (and evictions / next-layer starts / output stores) stagger ~1.7us
    apart instead of clustering at the layer end.
  - Tail floor: last MM -> half-split evict (ACT||DVE, ~0.45us) -> 2-way
    store (sync+scalar) -> HBM write receipt (~2us) -> TileContext
    drain/sem-clear/barriers (~1.2us) -> NRT's fixed ~6.4us end-of-execution
    postamble (measured invariant to teardown contents: skipping dma_reset
    or distributing the sem clears across sequencers changed nothing).
"""

import os
import sys

if "/opt/trn_rl_repo" not in sys.path:
    sys.path.insert(0, "/opt/trn_rl_repo")

import numpy as np

import concourse.bass as bass  # noqa: F401  (bass must import before mybir use)
import concourse.mybir as mybir
import concourse.tile as tile
from concourse import bacc
from concourse.bass_utils import run_bass_kernel_spmd

F32 = mybir.dt.float32
F16 = mybir.dt.float16
AF = mybir.ActivationFunctionType
ALU = mybir.AluOpType

WEXP = 8   # weight scale 2^8
ZEXP = 6   # blend scale 2^6
OEXP = 12  # output store scale 2^12 (fp16 out tiles hold true*2^12)
DESCALE = float(2.0 ** -(WEXP + ZEXP))

B, E = 4096, 8
DIMS = [512, 1024, 1024, 512]
N_CORES = 8
B_LOC = B // N_CORES  # 512; also the matmul moving free-dim
P = 128

# (in, out, apply_elu) per layer
LAYERS = [
    (DIMS[0], DIMS[1], True),
    (DIMS[1], DIMS[2], True),
    (DIMS[2], DIMS[3], False),
]

LAST_RESULTS = None  # BassKernelResults of the most recent run (for test.py)
_NC_CACHE = {}


def _build(has_bias):
    """Build the per-core module. has_bias=False (the case this problem's
    setup_inputs actually produces — all beta fills are zeros) drops the
    blended-bias matmuls and their beta/wb feeds entirely; each bank then
    closes on the last expert's product."""
    nc = bacc.Bacc(None, target_bir_lowering=False, debug=False)

    # DRAM inputs. Weights are HOST-PRESWIZZLED to [E, 128, ni*dout]:
    #   aW[l][e, p, j*dout + o] = a_l[e, o, j*128 + p] * 2^WEXP   (fp16)
    # so a per-expert DMA is one fully-contiguous 2D transfer.
    xt = nc.dram_tensor("xt", [P, (DIMS[0] // P) * B_LOC], F16, kind="ExternalInput")
    wbbd = nc.dram_tensor("wbb", [E, P, B_LOC], F16, kind="ExternalInput")
    aws = [
        nc.dram_tensor(f"aw{l}", [E, P, (din // P) * dout], F16, kind="ExternalInput")
        for l, (din, dout, _) in enumerate(LAYERS)
    ]
    wb, betas = None, []
    if has_bias:
        wb = nc.dram_tensor("wb", [E, B_LOC], F16, kind="ExternalInput")
        betas = [
            nc.dram_tensor(f"b{l}", [E, dout], F16, kind="ExternalInput")
            for l, (_, dout, _) in enumerate(LAYERS)
        ]
    outt = nc.dram_tensor("outt", [DIMS[3], B_LOC], F16, kind="ExternalOutput")

    with tile.TileContext(nc) as tc:
        with (
            tc.tile_pool(name="htp", bufs=12) as htp,
            tc.tile_pool(name="ztp", bufs=8) as ztp,
            tc.tile_pool(name="wp", bufs=4) as wp,
            tc.tile_pool(name="prep", bufs=4) as prep,
            tc.tile_pool(name="wbbp", bufs=8) as wbbp,
            tc.tile_pool(name="consts", bufs=1) as consts,
            tc.tile_pool(name="betap", bufs=2) as betap,
            tc.tile_pool(name="tmp", bufs=2) as tmp,
            tc.tile_pool(name="psp", bufs=8, space="PSUM") as psp,
        ):
            # --- startup ---
            # Critical path to the first real matmul: wbb[e0], xt[j0],
            # pre_w[j0]. All DMAs ride the two HWDGE rings (sync, scalar) —
            # the SWDGE (gpsimd) path has ~1us of Q7 descriptor emission per
            # transfer, which put wbb[e0] on the v2 critical path.
            # sync ring order: wbb0, pre_j0..3, then (w_e, wbb_e) interleaved.
            wbb = [None] * E
            wbb[0] = wbbp.tile([P, B_LOC], F16, tag="wbb", name="wbb0")
            nc.sync.dma_start(out=wbb[0], in_=wbbd[0, :, :])
            # j0's weights split 2:6 so the first two matmuls' 64KB chunk
            # lands ~0.6us before the rest
            preA = prep.tile([P, 2 * P], F16, tag="pwa", bufs=1)
            nc.sync.dma_start(out=preA, in_=aws[0][0, :, : 2 * P])
            # Startup feeds are DELIVERY-RATE bound: expert 0's phase needs
            # ~2.7MB inside its ~6.9us window (~HBM/NC ceiling), so the two
            # rings' per-deadline byte loads must be balanced (~<=200GB/s
            # each). Sync keeps preB/pre2/pre3; pre1 rides scalar between
            # xb and xc; e1's j2-3 weight half rides scalar's idle tail.
            preB = prep.tile([P, DIMS[1] - 2 * P], F16, tag="pwb", bufs=1)
            nc.sync.dma_start(out=preB, in_=aws[0][0, :, 2 * P : DIMS[1]])
            ht = []
            for j in range(4):
                t = htp.tile([P, B_LOC], F16, tag="ht", name=f"x{j}")
                nc.scalar.dma_start(out=t, in_=xt[:, j * B_LOC : (j + 1) * B_LOC])
                ht.append(t)
                if j == 1:
                    pre1 = prep.tile([P, DIMS[1]], F16, tag="pw", bufs=3)
                    nc.scalar.dma_start(
                        out=pre1, in_=aws[0][0, :, DIMS[1] : 2 * DIMS[1]]
                    )
            pre_w = [preB, pre1]
            for j in (2, 3):
                t = prep.tile([P, DIMS[1]], F16, tag="pw", bufs=3, name=f"pre{j}")
                nc.sync.dma_start(
                    out=t, in_=aws[0][0, :, j * DIMS[1] : (j + 1) * DIMS[1]]
                )
                pre_w.append(t)
            # wb as [E, B_LOC] tile: rhs of the (end-of-layer) bias matmuls
            wb_all = None
            if has_bias:
                wb_all = consts.tile([E, B_LOC], F16, tag="wb_all")
                nc.scalar.dma_start(out=wb_all, in_=wb[:, :])

            # PE warm-up: the HAM clock gate needs ~3.4us of sustained PE
            # activity to reach 2.4 GHz. Junk matmuls (cold: ~427ns each)
            # burn the startup DMA window so the real MMs start warm. The
            # junk memset rides gpsimd (its queue is otherwise empty), which
            # is live ~1us before the DVE at kernel start.
            junk = consts.tile([P, B_LOC], F16, tag="junk")
            nc.gpsimd.memset(junk, 0.0)
            warm_ps = psp.tile([P, B_LOC], F32, tag="ps")
            for _ in range(9):
                nc.tensor.matmul(warm_ps, junk[:, :P], junk, start=True, stop=True)

            # --- layers ---
            for l, (din, dout, use_act) in enumerate(LAYERS):
                ni, no = din // P, dout // P
                beta_sb = None
                if has_bias:
                    beta_sb = betap.tile([E, dout], F16, tag="beta")
                    nc.scalar.dma_start(out=beta_sb, in_=betas[l][:, :])

                psums = []
                for _ in range(no):
                    pt = psp.tile([P, B_LOC], F32, tag="ps", name="ps")
                    psums.append(pt)

                # expert weight tiles: one [128, ni*dout] DMA per expert on
                # the sync ring (e0 of L0 comes from the startup pre_w tiles)
                def w_slice(wt, j, c):
                    return wt[:, j * dout + c * P : j * dout + (c + 1) * P]

                wtiles = {}
                for e in range(E):
                    if l == 0 and e == 0:
                        continue
                    # blend-weight tile for this expert rides ahead of its
                    # weights on the same ring (l == 0 pass only): it's
                    # small and needed before the first j-tile is consumed
                    if l == 0:
                        t = wbbp.tile([P, B_LOC], F16, tag="wbb")
                        nc.sync.dma_start(out=t, in_=wbbd[e, :, :])
                        wbb[e] = t
                    wt = wp.tile([P, ni * dout], F16, tag="w")
                    if l == 0 and e == 1:
                        # e1 lands just-in-time behind the startup tiles on
                        # the sync ring; j0-j1 half stays sync (more margin
                        # with pre1 moved off), j2-j3 half rides scalar's
                        # idle tail
                        half = (ni // 2) * dout
                        nc.sync.dma_start(out=wt[:, :half], in_=aws[l][e, :, :half])
                        nc.scalar.dma_start(out=wt[:, half:], in_=aws[l][e, :, half:])
                    else:
                        nc.sync.dma_start(out=wt, in_=aws[l][e, :, :])
                    wtiles[e] = wt

                # accumulate experts 0..E-2 j-outer (consumes ht tiles as the
                # previous layer produces them; first expert opens each bank)
                for e in range(E - 1):
                    for j in range(ni):
                        zt = ztp.tile([P, B_LOC], F16, tag="zt")
                        if l == 0:
                            nc.vector.tensor_mul(zt, ht[j], wbb[e])
                        else:
                            # ht holds elu(x)+1; fold the -1 into the blend
                            nc.vector.scalar_tensor_tensor(
                                zt, ht[j], -1.0, wbb[e], ALU.add, ALU.mult
                            )
                        for c in range(no):
                            if l == 0 and e == 0:
                                if j == 0 and c < 2:
                                    lhsT = preA[:, c * P : (c + 1) * P]
                                elif j == 0:
                                    lhsT = pre_w[0][:, (c - 2) * P : (c - 1) * P]
                                else:
                                    lhsT = pre_w[j][:, c * P : (c + 1) * P]
                            else:
                                lhsT = w_slice(wtiles[e], j, c)
                            nc.tensor.matmul(
                                psums[c],
                                lhsT,
                                zt,
                                start=(e == 0 and j == 0),
                                stop=False,
                            )
                # last expert runs c-outer (bank-by-bank) so bank closures —
                # and therefore evictions, next-layer bank reuse, and the
                # final output stores — spread across the last ~ni*no matmuls
                # instead of clustering after the end.
                e = E - 1
                zts = []
                for j in range(ni):
                    zt = ztp.tile([P, B_LOC], F16, tag="zt")
                    if l == 0:
                        nc.vector.tensor_mul(zt, ht[j], wbb[e])
                    else:
                        nc.vector.scalar_tensor_tensor(
                            zt, ht[j], -1.0, wbb[e], ALU.add, ALU.mult
                        )
                    zts.append(zt)
                halved = (not use_act) and (not has_bias)
                for c in range(no):
                    if halved:
                        # final layer: close each bank in batch-HALVES
                        # (N=256 keeps the MM issue rate amortized) so the
                        # very last eviction+store chain covers 256 cols,
                        # whichever closure the scheduler runs last
                        for h in range(2):
                            hs = slice(h * (B_LOC // 2), (h + 1) * (B_LOC // 2))
                            for j in range(ni):
                                nc.tensor.matmul(
                                    psums[c][:, hs],
                                    w_slice(wtiles[e], j, c),
                                    zts[j][:, hs],
                                    start=False,
                                    stop=(j == ni - 1),
                                )
                        continue
                    for j in range(ni):
                        nc.tensor.matmul(
                            psums[c],
                            w_slice(wtiles[e], j, c),
                            zts[j],
                            start=False,
                            stop=(not has_bias and j == ni - 1),
                        )
                    if has_bias:
                        nc.tensor.matmul(
                            psums[c],
                            beta_sb[:, c * P : (c + 1) * P],
                            wb_all,
                            start=False,
                            stop=True,
                        )

                # evict: elu(x)+1 for layers 0/1, scaled-fp16 DMA out for l 2
                if use_act:
                    new_ht = []
                    for c in range(no):
                        r = tmp.tile([P, B_LOC], F32, tag="relu")
                        x = tmp.tile([P, B_LOC], F32, tag="expz")
                        h = htp.tile([P, B_LOC], F32, tag="ht")
                        nc.scalar.activation(r, psums[c], AF.Relu, scale=DESCALE)
                        nc.scalar.activation(x, psums[c], AF.Exp, scale=DESCALE)
                        # h = min(x, 1) + r  ( = elu + 1 )
                        nc.vector.scalar_tensor_tensor(h, x, 1.0, r, ALU.min, ALU.add)
                        new_ht.append(h)
                    ht = new_ht
                else:
                    oscale = float(DESCALE * (2.0**OEXP))
                    rings = [nc.sync, nc.scalar]
                    for c in range(no):
                        o = tmp.tile([P, B_LOC], F16, tag="out")
                        if halved:
                            # evict+store each half as it closes: half A on
                            # ACT, half B on DVE; stores alternate rings so
                            # no sequencer queues two 0.6us issues back to
                            # back at the kernel end
                            for h in range(2):
                                hs = slice(
                                    h * (B_LOC // 2), (h + 1) * (B_LOC // 2)
                                )
                                if h == 0:
                                    nc.scalar.activation(
                                        o[:, hs], psums[c][:, hs], AF.Copy,
                                        scale=oscale,
                                    )
                                else:
                                    nc.vector.tensor_scalar_mul(
                                        o[:, hs], psums[c][:, hs], oscale
                                    )
                                rings[(2 * c + h) % 2].dma_start(
                                    out=outt[c * P : (c + 1) * P, hs],
                                    in_=o[:, hs],
                                )
                            continue
                        # (has_bias fallback: full-width banks)
                        nc.scalar.activation(
                            o[:, : B_LOC // 2], psums[c][:, : B_LOC // 2],
                            AF.Copy, scale=oscale,
                        )
                        nc.vector.tensor_scalar_mul(
                            o[:, B_LOC // 2 :], psums[c][:, B_LOC // 2 :], oscale
                        )
                        last = c == no - 1
                        if not last:
                            nc.sync.dma_start(
                                out=outt[c * P : (c + 1) * P, :], in_=o
                            )
                        else:
                            step = P // 2
                            for q in range(2):
                                rings[q].dma_start(
                                    out=outt[
                                        c * P + q * step : c * P + (q + 1) * step, :
                                    ],
                                    in_=o[q * step : (q + 1) * step, :],
                                )

    nc.compile()
    return nc


def _maybe_reset_device():
    """Clear stale NRT state on the axon terminal left by a crashed prior
    process. Only safe/needed before this process initializes its jax
    backend, and must run in a subprocess (CDLL'ing the axon .so in-process
    conflicts with jax's own dlopen)."""
    try:
        import jax._src.xla_bridge as xb

        if getattr(xb, "_backends", None):
            return  # backend already live in this process; don't touch it
    except Exception:
        pass
    try:
        import subprocess

        subprocess.run(
            [
                sys.executable,
                "-c",
                "import ctypes; lib = ctypes.CDLL('/opt/axon/libaxon_pjrt.so'); "
                "lib.axon_reset.restype = ctypes.c_int64; lib.axon_reset()",
            ],
            timeout=60,
            capture_output=True,
        )
    except Exception:
        pass


def kernel(x, weight_blend, a0, b0, a1, b1, a2, b2):
    global LAST_RESULTS, _NC_CACHE
    _maybe_reset_device()
    x = np.asarray(x, dtype=np.float32)
    weight_blend = np.ascontiguousarray(np.asarray(weight_blend, dtype=np.float32))
    # Host weight prep: transpose to [E, din, dout], scale to fp16 range,
    # then swizzle to the SBUF layout [E, 128, ni*dout] (partition p holds
    # row j*128+p of aT for every j, concatenated along the free dim).
    aWs = []
    for a in (a0, a1, a2):
        aT = np.asarray(a, dtype=np.float32).transpose(0, 2, 1) * float(2.0**WEXP)
        e_, din_, dout_ = aT.shape
        ni = din_ // P
        aW = np.ascontiguousarray(
            aT.reshape(e_, ni, P, dout_).transpose(0, 2, 1, 3).reshape(e_, P, ni * dout_)
        ).astype(np.float16)
        aWs.append(aW)
    bs = [
        np.ascontiguousarray(
            (np.asarray(b, dtype=np.float32) * float(2.0**WEXP)).astype(np.float16)
        )
        for b in (b0, b1, b2)
    ]
    has_bias = any(np.any(b) for b in bs)

    if has_bias not in _NC_CACHE:
        _NC_CACHE[has_bias] = _build(has_bias)
    nc = _NC_CACHE[has_bias]

    in_maps = []
    for cix in range(N_CORES):
        sl = slice(cix * B_LOC, (cix + 1) * B_LOC)
        wb_c = weight_blend[:, sl] * float(2.0**ZEXP)
        # xt swizzled like the weights: [128, 4*512], partition p holds
        # x[sl][:, j*128+p] for every j
        xT = x[sl].T  # [512, 512] = [din, b]
        xW = np.ascontiguousarray(
            xT.reshape(4, P, B_LOC).transpose(1, 0, 2).reshape(P, 4 * B_LOC)
        ).astype(np.float16)
        m = {
            "xt": xW,
            "wbb": np.ascontiguousarray(
                np.broadcast_to(wb_c[:, None, :], (E, P, B_LOC))
            ).astype(np.float16),
            "aw0": aWs[0],
            "aw1": aWs[1],
            "aw2": aWs[2],
        }
        if has_bias:
            m["wb"] = wb_c.astype(np.float16)
            m["b0"], m["b1"], m["b2"] = bs
        in_maps.append(m)

    trace = os.environ.get("BASS_KERNEL_TRACE") == "1"
    res = run_bass_kernel_spmd(
        nc, in_maps, core_ids=list(range(N_CORES)), trace=trace
    )
    LAST_RESULTS = res
    return np.concatenate(
        [
            np.asarray(r["outt"]).T.astype(np.float32) * float(2.0**-OEXP)
            for r in res.results
        ],
        axis=0,
    )


# revision 44
# speedup vs baseline: 1.0048x; 1.0048x over previous
"""MoE soft-routing MLP kernel for 8 Trainium2 NeuronCores.

Reference computation (per layer l, weights a_l: [E, out, in], bias b_l: [E, out]):
    y_e = H @ a_e^T + b_e          # per-expert GEMM      [B, out]
    H'  = sum_e wb[e, :, None] * y_e                      [B, out]
    H'  = elu(H') for layers 0, 1

Distribution: data-parallel over batch B=4096 across 8 cores (B_loc=512).
Expert weights are replicated to every core; x and weight_blend are sharded
along batch.

Per-core algorithm (all activations kept TRANSPOSED on chip: [feature, batch]):
    out[o, b] = sum_e sum_i aT_e[i, o] * (wb[e, b] * Ht[i, b])  + bias term
  - each expert's contribution accumulates into the same PSUM bank:
    lhsT = aT_e[i-tile, o-chunk] (128x128 stationary),
    rhs  = Zt_e[i-tile] = Ht[i-tile] * bcast(wb[e, :]) (128x512 moving),
  - blend weights arrive pre-broadcast from the host ([E, 128, B_LOC] fp16).
  - ELU+1 is evicted as relu(x) + min(exp(x), 1)  (= elu(x) + 1; valid since
    the preactivations here are far below exp-overflow), and the -1 folds
    into the next layer's blend: zt = (h1 - 1) * wbb_e, one DVE op.
    (ht must stay fp32: elu values are ~1e-2 around the +1 offset, so fp16
    storage of elu+1 would cost ~3% relative error after the -1.)

Matmul operands are fp16 with fp32 PSUM accumulation (1 PE cycle/row and
half the weight-DMA bytes of fp32; weights are pre-scaled by 2^8 and blend
weights by 2^6 on the host so fp16 products stay clear of the subnormal
range; the 2^-14 descale folds into the PSUM-eviction activations).
PE floor: 1024 matmuls x ~216ns = ~221us; DMA total ~20MB/core.

Scheduling design (from trace analysis; measured ~239.6us/core vs a
~221.4us pure-PE floor, with ZERO PE gaps in steady state):
  - Weights live in DRAM pre-swizzled to SBUF layout [E, 128, ni*dout]
    (host does the permutation), so ONE dma_start per expert per layer
    moves the whole expert contiguously (8-16KB/partition descriptors) at
    near-HBM rate. ~47 dma_starts total: DMA *issue* time (~0.6us each on
    a sequencer) stops gating the startup.
  - Everything rides the two HWDGE rings (sync / scalar) — the SWDGE
    (gpsimd) path adds ~1us of Q7 descriptor emission, which put the
    blend-weight broadcast on the v2 critical path. Startup-critical order:
    sync: wbb[e0], w[e0] j0 split 2:6, j1..j3, then (wbb[e], w[e]) pairs;
    scalar: the four xt j-tiles, later the output stores.
  - x and the blend broadcast are fp16 (x is N(0,1): 5e-4 quantization,
    same as the matmul operands already pay). Output is stored fp16 scaled
    by 2^12 (values ~0.27) and un-scaled on the host: halves the tail DMA.
  - 9 junk matmuls bridge the ~4.5us from kernel start to the first
    operand landing (HWDGE issue + HBM read + write-receipt ~4us for the
    first tiles) and warm the PE HAM clock gate (~3.4us sustained activity
    -> 2.4GHz) so the real MMs run warm from the first one.
  - The last-expert pass of each layer runs c-outer so PSUM bank closures
    (and evictions / next-layer starts / output stores) stagger ~1.7us
    apart instead of clustering at the layer end.
  - Tail floor: last MM -> half-split evict (ACT||DVE, ~0.45us) -> 2-way
    store (sync+scalar) -> HBM write receipt (~2us) -> TileContext
    drain/sem-clear/barriers (~1.2us) -> NRT's fixed ~6.4us end-of-execution
    postamble (measured invariant to teardown contents: skipping dma_reset
    or distributing the sem clears across sequencers changed nothing).
"""

import os
import sys

if "/opt/trn_rl_repo" not in sys.path:
    sys.path.insert(0, "/opt/trn_rl_repo")

import numpy as np

import concourse.bass as bass  # noqa: F401  (bass must import before mybir use)
import concourse.mybir as mybir
import concourse.tile as tile
from concourse import bacc
from concourse.bass_utils import run_bass_kernel_spmd

F32 = mybir.dt.float32
F16 = mybir.dt.float16
AF = mybir.ActivationFunctionType
ALU = mybir.AluOpType

WEXP = 8   # weight scale 2^8
ZEXP = 6   # blend scale 2^6
OEXP = 12  # output store scale 2^12 (fp16 out tiles hold true*2^12)
DESCALE = float(2.0 ** -(WEXP + ZEXP))

B, E = 4096, 8
DIMS = [512, 1024, 1024, 512]
N_CORES = 8
B_LOC = B // N_CORES  # 512; also the matmul moving free-dim
P = 128

# (in, out, apply_elu) per layer
LAYERS = [
    (DIMS[0], DIMS[1], True),
    (DIMS[1], DIMS[2], True),
    (DIMS[2], DIMS[3], False),
]

LAST_RESULTS = None  # BassKernelResults of the most recent run (for test.py)
_NC_CACHE = {}


def _build(has_bias):
    """Build the per-core module. has_bias=False (the case this problem's
    setup_inputs actually produces — all beta fills are zeros) drops the
    blended-bias matmuls and their beta/wb feeds entirely; each bank then
    closes on the last expert's product."""
    nc = bacc.Bacc(None, target_bir_lowering=False, debug=False)

    # DRAM inputs. Weights are HOST-PRESWIZZLED to [E, 128, ni*dout]:
    #   aW[l][e, p, j*dout + o] = a_l[e, o, j*128 + p] * 2^WEXP   (fp16)
    # so a per-expert DMA is one fully-contiguous 2D transfer.
    xt = nc.dram_tensor("xt", [P, (DIMS[0] // P) * B_LOC], F16, kind="ExternalInput")
    wbbd = nc.dram_tensor("wbb", [E, P, B_LOC], F16, kind="ExternalInput")
    aws = [
        nc.dram_tensor(f"aw{l}", [E, P, (din // P) * dout], F16, kind="ExternalInput")
        for l, (din, dout, _) in enumerate(LAYERS)
    ]
    wb, betas = None, []
    if has_bias:
        wb = nc.dram_tensor("wb", [E, B_LOC], F16, kind="ExternalInput")
        betas = [
            nc.dram_tensor(f"b{l}", [E, dout], F16, kind="ExternalInput")
            for l, (_, dout, _) in enumerate(LAYERS)
        ]
    outt = nc.dram_tensor("outt", [DIMS[3], B_LOC], F16, kind="ExternalOutput")

    with tile.TileContext(nc) as tc:
        with (
            tc.tile_pool(name="htp", bufs=12) as htp,
            tc.tile_pool(name="ztp", bufs=8) as ztp,
            tc.tile_pool(name="wp", bufs=4) as wp,
            tc.tile_pool(name="prep", bufs=4) as prep,
            tc.tile_pool(name="wbbp", bufs=8) as wbbp,
            tc.tile_pool(name="consts", bufs=1) as consts,
            tc.tile_pool(name="betap", bufs=2) as betap,
            tc.tile_pool(name="tmp", bufs=2) as tmp,
            tc.tile_pool(name="psp", bufs=8, space="PSUM") as psp,
        ):
            # --- startup ---
            # Critical path to the first real matmul: wbb[e0], xt[j0],
            # pre_w[j0]. All DMAs ride the two HWDGE rings (sync, scalar) —
            # the SWDGE (gpsimd) path has ~1us of Q7 descriptor emission per
            # transfer, which put wbb[e0] on the v2 critical path.
            # sync ring order: wbb0, pre_j0..3, then (w_e, wbb_e) interleaved.
            wbb = [None] * E
            wbb[0] = wbbp.tile([P, B_LOC], F16, tag="wbb", name="wbb0")
            nc.sync.dma_start(out=wbb[0], in_=wbbd[0, :, :])
            # j0's weights split 2:6 so the first two matmuls' 64KB chunk
            # lands ~0.6us before the rest
            preA = prep.tile([P, 2 * P], F16, tag="pwa", bufs=1)
            nc.sync.dma_start(out=preA, in_=aws[0][0, :, : 2 * P])
            pre_w = [None]
            for j in range(DIMS[0] // P):
                if j == 0:
                    t = prep.tile([P, DIMS[1] - 2 * P], F16, tag="pwb", bufs=1)
                    nc.sync.dma_start(out=t, in_=aws[0][0, :, 2 * P : DIMS[1]])
                    pre_w[0] = t
                else:
                    t = prep.tile([P, DIMS[1]], F16, tag="pw", bufs=3)
                    nc.sync.dma_start(
                        out=t, in_=aws[0][0, :, j * DIMS[1] : (j + 1) * DIMS[1]]
                    )
                    pre_w.append(t)
            # x^T tiles (scalar ring): j0 and j1 alone so their z start early.
            ht = []
            for j in range(4):
                t = htp.tile([P, B_LOC], F16, tag="ht", name=f"x{j}")
                nc.scalar.dma_start(out=t, in_=xt[:, j * B_LOC : (j + 1) * B_LOC])
                ht.append(t)
            # wb as [E, B_LOC] tile: rhs of the (end-of-layer) bias matmuls
            wb_all = None
            if has_bias:
                wb_all = consts.tile([E, B_LOC], F16, tag="wb_all")
                nc.scalar.dma_start(out=wb_all, in_=wb[:, :])

            # PE warm-up: the HAM clock gate needs ~3.4us of sustained PE
            # activity to reach 2.4 GHz. Junk matmuls (cold: ~427ns each)
            # burn the startup DMA window so the real MMs start warm. The
            # junk memset rides gpsimd (its queue is otherwise empty), which
            # is live ~1us before the DVE at kernel start.
            junk = consts.tile([P, B_LOC], F16, tag="junk")
            nc.gpsimd.memset(junk, 0.0)
            warm_ps = psp.tile([P, B_LOC], F32, tag="ps")
            for _ in range(9):
                nc.tensor.matmul(warm_ps, junk[:, :P], junk, start=True, stop=True)

            # --- layers ---
            for l, (din, dout, use_act) in enumerate(LAYERS):
                ni, no = din // P, dout // P
                beta_sb = None
                if has_bias:
                    beta_sb = betap.tile([E, dout], F16, tag="beta")
                    nc.scalar.dma_start(out=beta_sb, in_=betas[l][:, :])

                psums = []
                for _ in range(no):
                    pt = psp.tile([P, B_LOC], F32, tag="ps", name="ps")
                    psums.append(pt)

                # expert weight tiles: one [128, ni*dout] DMA per expert on
                # the sync ring (e0 of L0 comes from the startup pre_w tiles)
                def w_slice(wt, j, c):
                    return wt[:, j * dout + c * P : j * dout + (c + 1) * P]

                wtiles = {}
                for e in range(E):
                    if l == 0 and e == 0:
                        continue
                    # blend-weight tile for this expert rides ahead of its
                    # weights on the same ring (l == 0 pass only): it's
                    # small and needed before the first j-tile is consumed
                    if l == 0:
                        t = wbbp.tile([P, B_LOC], F16, tag="wbb")
                        nc.sync.dma_start(out=t, in_=wbbd[e, :, :])
                        wbb[e] = t
                    wt = wp.tile([P, ni * dout], F16, tag="w")
                    if l == 0 and e == 1:
                        # e1 lands just-in-time behind the ~1.2MB of startup
                        # tiles on the ring; split it so the j0/j1 half gets
                        # ~1us more margin against HBM latency jitter
                        half = (ni // 2) * dout
                        nc.sync.dma_start(out=wt[:, :half], in_=aws[l][e, :, :half])
                        nc.sync.dma_start(out=wt[:, half:], in_=aws[l][e, :, half:])
                    else:
                        nc.sync.dma_start(out=wt, in_=aws[l][e, :, :])
                    wtiles[e] = wt

                # accumulate experts 0..E-2 j-outer (consumes ht tiles as the
                # previous layer produces them; first expert opens each bank)
                for e in range(E - 1):
                    for j in range(ni):
                        zt = ztp.tile([P, B_LOC], F16, tag="zt")
                        if l == 0:
                            nc.vector.tensor_mul(zt, ht[j], wbb[e])
                        else:
                            # ht holds elu(x)+1; fold the -1 into the blend
                            nc.vector.scalar_tensor_tensor(
                                zt, ht[j], -1.0, wbb[e], ALU.add, ALU.mult
                            )
                        for c in range(no):
                            if l == 0 and e == 0:
                                if j == 0 and c < 2:
                                    lhsT = preA[:, c * P : (c + 1) * P]
                                elif j == 0:
                                    lhsT = pre_w[0][:, (c - 2) * P : (c - 1) * P]
                                else:
                                    lhsT = pre_w[j][:, c * P : (c + 1) * P]
                            else:
                                lhsT = w_slice(wtiles[e], j, c)
                            nc.tensor.matmul(
                                psums[c],
                                lhsT,
                                zt,
                                start=(e == 0 and j == 0),
                                stop=False,
                            )
                # last expert runs c-outer (bank-by-bank) so bank closures —
                # and therefore evictions, next-layer bank reuse, and the
                # final output stores — spread across the last ~ni*no matmuls
                # instead of clustering after the end.
                e = E - 1
                zts = []
                for j in range(ni):
                    zt = ztp.tile([P, B_LOC], F16, tag="zt")
                    if l == 0:
                        nc.vector.tensor_mul(zt, ht[j], wbb[e])
                    else:
                        nc.vector.scalar_tensor_tensor(
                            zt, ht[j], -1.0, wbb[e], ALU.add, ALU.mult
                        )
                    zts.append(zt)
                halved = (not use_act) and (not has_bias)
                for c in range(no):
                    if halved:
                        # final layer: close each bank in batch-HALVES
                        # (N=256 keeps the MM issue rate amortized) so the
                        # very last eviction+store chain covers 256 cols,
                        # whichever closure the scheduler runs last
                        for h in range(2):
                            hs = slice(h * (B_LOC // 2), (h + 1) * (B_LOC // 2))
                            for j in range(ni):
                                nc.tensor.matmul(
                                    psums[c][:, hs],
                                    w_slice(wtiles[e], j, c),
                                    zts[j][:, hs],
                                    start=False,
                                    stop=(j == ni - 1),
                                )
                        continue
                    for j in range(ni):
                        nc.tensor.matmul(
                            psums[c],
                            w_slice(wtiles[e], j, c),
                            zts[j],
                            start=False,
                            stop=(not has_bias and j == ni - 1),
                        )
                    if has_bias:
                        nc.tensor.matmul(
                            psums[c],
                            beta_sb[:, c * P : (c + 1) * P],
                            wb_all,
                            start=False,
                            stop=True,
                        )

                # evict: elu(x)+1 for layers 0/1, scaled-fp16 DMA out for l 2
                if use_act:
                    new_ht = []
                    for c in range(no):
                        r = tmp.tile([P, B_LOC], F32, tag="relu")
                        x = tmp.tile([P, B_LOC], F32, tag="expz")
                        h = htp.tile([P, B_LOC], F32, tag="ht")
                        nc.scalar.activation(r, psums[c], AF.Relu, scale=DESCALE)
                        nc.scalar.activation(x, psums[c], AF.Exp, scale=DESCALE)
                        # h = min(x, 1) + r  ( = elu + 1 )
                        nc.vector.scalar_tensor_tensor(h, x, 1.0, r, ALU.min, ALU.add)
                        new_ht.append(h)
                    ht = new_ht
                else:
                    oscale = float(DESCALE * (2.0**OEXP))
                    rings = [nc.sync, nc.scalar]
                    for c in range(no):
                        o = tmp.tile([P, B_LOC], F16, tag="out")
                        if halved:
                            # evict+store each half as it closes: half A on
                            # ACT, half B on DVE; stores alternate rings so
                            # no sequencer queues two 0.6us issues back to
                            # back at the kernel end
                            for h in range(2):
                                hs = slice(
                                    h * (B_LOC // 2), (h + 1) * (B_LOC // 2)
                                )
                                if h == 0:
                                    nc.scalar.activation(
                                        o[:, hs], psums[c][:, hs], AF.Copy,
                                        scale=oscale,
                                    )
                                else:
                                    nc.vector.tensor_scalar_mul(
                                        o[:, hs], psums[c][:, hs], oscale
                                    )
                                rings[(2 * c + h) % 2].dma_start(
                                    out=outt[c * P : (c + 1) * P, hs],
                                    in_=o[:, hs],
                                )
                            continue
                        # (has_bias fallback: full-width banks)
                        nc.scalar.activation(
                            o[:, : B_LOC // 2], psums[c][:, : B_LOC // 2],
                            AF.Copy, scale=oscale,
                        )
                        nc.vector.tensor_scalar_mul(
                            o[:, B_LOC // 2 :], psums[c][:, B_LOC // 2 :], oscale
                        )
                        last = c == no - 1
                        if not last:
                            nc.sync.dma_start(
                                out=outt[c * P : (c + 1) * P, :], in_=o
                            )
                        else:
                            step = P // 2
                            for q in range(2):
                                rings[q].dma_start(
                                    out=outt[
                                        c * P + q * step : c * P + (q + 1) * step, :
                                    ],
                                    in_=o[q * step : (q + 1) * step, :],
                                )

    nc.compile()
    return nc


def _maybe_reset_device():
    """Clear stale NRT state on the axon terminal left by a crashed prior
    process. Only safe/needed before this process initializes its jax
    backend, and must run in a subprocess (CDLL'ing the axon .so in-process
    conflicts with jax's own dlopen)."""
    try:
        import jax._src.xla_bridge as xb

        if getattr(xb, "_backends", None):
            return  # backend already live in this process; don't touch it
    except Exception:
        pass
    try:
        import subprocess

        subprocess.run(
            [
                sys.executable,
                "-c",
                "import ctypes; lib = ctypes.CDLL('/opt/axon/libaxon_pjrt.so'); "
                "lib.axon_reset.restype = ctypes.c_int64; lib.axon_reset()",
            ],
            timeout=60,
            capture_output=True,
        )
    except Exception:
        pass


def kernel(x, weight_blend, a0, b0, a1, b1, a2, b2):
    global LAST_RESULTS, _NC_CACHE
    _maybe_reset_device()
    x = np.asarray(x, dtype=np.float32)
    weight_blend = np.ascontiguousarray(np.asarray(weight_blend, dtype=np.float32))
    # Host weight prep: transpose to [E, din, dout], scale to fp16 range,
    # then swizzle to the SBUF layout [E, 128, ni*dout] (partition p holds
    # row j*128+p of aT for every j, concatenated along the free dim).
    aWs = []
    for a in (a0, a1, a2):
        aT = np.asarray(a, dtype=np.float32).transpose(0, 2, 1) * float(2.0**WEXP)
        e_, din_, dout_ = aT.shape
        ni = din_ // P
        aW = np.ascontiguousarray(
            aT.reshape(e_, ni, P, dout_).transpose(0, 2, 1, 3).reshape(e_, P, ni * dout_)
        ).astype(np.float16)
        aWs.append(aW)
    bs = [
        np.ascontiguousarray(
            (np.asarray(b, dtype=np.float32) * float(2.0**WEXP)).astype(np.float16)
        )
        for b in (b0, b1, b2)
    ]
    has_bias = any(np.any(b) for b in bs)

    if has_bias not in _NC_CACHE:
        _NC_CACHE[has_bias] = _build(has_bias)
    nc = _NC_CACHE[has_bias]

    in_maps = []
    for cix in range(N_CORES):
        sl = slice(cix * B_LOC, (cix + 1) * B_LOC)
        wb_c = weight_blend[:, sl] * float(2.0**ZEXP)
        # xt swizzled like the weights: [128, 4*512], partition p holds
        # x[sl][:, j*128+p] for every j
        xT = x[sl].T  # [512, 512] = [din, b]
        xW = np.ascontiguousarray(
            xT.reshape(4, P, B_LOC).transpose(1, 0, 2).reshape(P, 4 * B_LOC)
        ).astype(np.float16)
        m = {
            "xt": xW,
            "wbb": np.ascontiguousarray(
                np.broadcast_to(wb_c[:, None, :], (E, P, B_LOC))
            ).astype(np.float16),
            "aw0": aWs[0],
            "aw1": aWs[1],
            "aw2": aWs[2],
        }
        if has_bias:
            m["wb"] = wb_c.astype(np.float16)
            m["b0"], m["b1"], m["b2"] = bs
        in_maps.append(m)

    trace = os.environ.get("BASS_KERNEL_TRACE") == "1"
    res = run_bass_kernel_spmd(
        nc, in_maps, core_ids=list(range(N_CORES)), trace=trace
    )
    LAST_RESULTS = res
    return np.concatenate(
        [
            np.asarray(r["outt"]).T.astype(np.float32) * float(2.0**-OEXP)
            for r in res.results
        ],
        axis=0,
    )
